# revision 4
# baseline (speedup 1.0000x reference)
"""Trainium2 Bass kernel for nn_AttentionMask (scatter_memory).

Semantics (from the reference):
  - coords_x is the deterministic lexicographic grid (batch 0, z = i//16384,
    y = (i//128)%128, x = i%128) so the packed hash key of row i is strictly
    increasing in i: searchsorted(keys_x_sorted, key_m) degenerates to the
    linear index row = z*16384 + y*128 + x, and "found" is a simple range
    check on the coordinate fields.
  - mask scores are scatter-added onto their x rows; a row survives iff the
    accumulated score truncates (float->int) to nonzero, i.e. sum >= 1.0
    (scores are non-negative).
  - output = (feats_x zero-masked by survival, survival mask as bool).

Distribution: rows of x are sharded across 8 cores (250k rows each).  The
host packs+sorts the mask points by target row (as blessed by the problem's
sharding hint: "broadcast the sorted coordinate hash keys of the mask") and
routes each core/partition its slice.  Each core then, on device:
  1. merges duplicate rows with a segmented scan (DVE tensor_tensor_scan:
     state = same_as_prev*state + val, so the run-sum sits at each run tail),
  2. scatters the run-sums into a dense per-partition accumulator with the
     GPSIMD local_scatter vector-scatter op (f32 values travel as two uint16
     bit-planes since local_scatter moves 2-byte elements),
  3. thresholds (>= 1.0) to build the survival mask,
  4. streams feats_x through SBUF multiplying by the broadcast mask.
"""

import numpy as np

import concourse.bass as bass
import concourse.mybir as mybir
import concourse.tile as tile
from concourse import bacc
from concourse.bass_utils import run_bass_kernel_spmd

NX = 2_000_000
CF = 16
NCORES = 8
RPC = 250_000          # real rows per core
E = 1954               # accumulator elements per partition (128*E = 250112 >= RPC)
PADR = 128 * E         # padded rows per core
C = 2048               # point capacity per partition (>= E covers any deduped input)

PRE = -1.0e6           # sentinel before column 0 (forces a run head)
PAD = -2.0e6           # padding "row" for unused point slots
SUF = -3.0e6           # sentinel after the last column (forces a run tail)

F32 = mybir.dt.float32
I16 = mybir.dt.int16
U16 = mybir.dt.uint16
U8 = mybir.dt.uint8


def _chunks():
    # split E row-groups into 8 DMA chunks of ~equal size
    base = E // 8
    rem = E - base * 8
    out = []
    g0 = 0
    for i in range(8):
        rg = base + (1 if i < rem else 0)
        out.append((g0, rg))
        g0 += rg
    return out


def build_device_program(nc, tc, ctx):
    feats = nc.dram_tensor("feats", [PADR, CF], F32, kind="ExternalInput")
    rows_p = nc.dram_tensor("rows", [128, C + 2], F32, kind="ExternalInput")
    vals_p = nc.dram_tensor("vals", [128, C], F32, kind="ExternalInput")
    pbase = nc.dram_tensor("pbase", [128, 1], F32, kind="ExternalInput")
    outf = nc.dram_tensor("out", [PADR, CF], F32, kind="ExternalOutput")
    targ = nc.dram_tensor("targ", [PADR], U8, kind="ExternalOutput")

    pool = ctx.enter_context(tc.tile_pool(name="prep", bufs=1))
    fpool = ctx.enter_context(tc.tile_pool(name="feats", bufs=4))

    rows_t = pool.tile([128, C + 2], F32)
    vals_t = pool.tile([128, C], F32)
    base_t = pool.tile([128, 1], F32)
    nc.sync.dma_start(rows_t[:], rows_p[:])
    nc.sync.dma_start(vals_t[:], vals_p[:])
    nc.sync.dma_start(base_t[:], pbase[:])

    r_cur = rows_t[:, 1 : C + 1]

    # same-as-previous flags and run-tail flags
    m_t = pool.tile([128, C], F32)
    nc.vector.tensor_tensor(m_t[:], r_cur, rows_t[:, 0:C], mybir.AluOpType.is_equal)
    t_t = pool.tile([128, C], F32)
    nc.vector.tensor_tensor(
        t_t[:], r_cur, rows_t[:, 2 : C + 2], mybir.AluOpType.not_equal
    )

    # segmented run sums: state = m*state + val  (run total lands on the tail)
    s_t = pool.tile([128, C], F32)
    nc.vector.tensor_tensor_scan(
        s_t[:], m_t[:], vals_t[:], 0.0, mybir.AluOpType.mult, mybir.AluOpType.add
    )

    # local index at run tails, -1 elsewhere: idx = max((row - base + 1)*T - 1, -1)
    idx_f = pool.tile([128, C], F32)
    nc.vector.tensor_tensor(
        idx_f[:], r_cur, base_t[:].to_broadcast([128, C]), mybir.AluOpType.subtract
    )
    nc.vector.tensor_scalar(
        idx_f[:], idx_f[:], 1.0, None, mybir.AluOpType.add
    )
    nc.vector.tensor_tensor(idx_f[:], idx_f[:], t_t[:], mybir.AluOpType.mult)
    nc.vector.tensor_scalar(
        idx_f[:], idx_f[:], -1.0, -1.0, mybir.AluOpType.add, mybir.AluOpType.max
    )
    idx_t = pool.tile([128, C], I16)
    nc.vector.tensor_copy(idx_t[:], idx_f[:])

    # split f32 run-sums into two uint16 bit-planes (little-endian lo/hi)
    s_bits = s_t[:].bitcast(U16).rearrange("p (n two) -> p n two", two=2)
    lo_t = pool.tile([128, C], U16)
    hi_t = pool.tile([128, C], U16)
    nc.vector.tensor_copy(lo_t[:].unsqueeze(2), s_bits[:, :, 0:1])
    nc.vector.tensor_copy(hi_t[:].unsqueeze(2), s_bits[:, :, 1:2])

    # dense per-partition scatter of both planes (zeroes the destinations)
    dst_lo = pool.tile([128, E], U16)
    dst_hi = pool.tile([128, E], U16)
    nc.gpsimd.local_scatter(dst_lo[:], lo_t[:], idx_t[:], 128, E, C)
    nc.gpsimd.local_scatter(dst_hi[:], hi_t[:], idx_t[:], 128, E, C)

    # reinterleave planes -> f32 accumulator, then threshold
    comb = pool.tile([128, E, 2], U16)
    nc.vector.tensor_copy(comb[:, :, 0:1], dst_lo[:].unsqueeze(2))
    nc.vector.tensor_copy(comb[:, :, 1:2], dst_hi[:].unsqueeze(2))
    acc_f = comb[:].rearrange("p n two -> p (n two)").bitcast(F32)

    mask_t = pool.tile([128, E], F32)
    nc.vector.tensor_scalar(
        mask_t[:], acc_f, 1.0, None, mybir.AluOpType.is_ge
    )
    targ_t = pool.tile([128, E], U8)
    nc.vector.tensor_copy(targ_t[:], mask_t[:])
    nc.sync.dma_start(targ[:].rearrange("(p n) -> p n", p=128), targ_t[:])

    # main memory-bound loop: out = feats * mask (mask broadcast over CF)
    fview = feats[:].rearrange("(p n) f -> p n f", p=128)
    oview = outf[:].rearrange("(p n) f -> p n f", p=128)
    maxrg = max(rg for _, rg in _chunks())
    for g0, rg in _chunks():
        ft_full = fpool.tile([128, maxrg, CF], F32, tag="ft")
        ft = ft_full[:, :rg, :]
        nc.sync.dma_start(ft, fview[:, g0 : g0 + rg, :])
        mb = mask_t[:, g0 : g0 + rg].unsqueeze(2).to_broadcast([128, rg, CF])
        nc.vector.tensor_tensor(ft, ft, mb, mybir.AluOpType.mult)
        nc.sync.dma_start(oview[:, g0 : g0 + rg, :], ft)


_CACHED_NC = None


def _get_program():
    global _CACHED_NC
    if _CACHED_NC is None:
        from contextlib import ExitStack

        nc = bacc.Bacc(
            "TRN2", target_bir_lowering=False, debug=False, num_devices=NCORES
        )
        with tile.TileContext(nc) as tc:
            with ExitStack() as ctx:
                build_device_program(nc, tc, ctx)
        nc.compile()
        _CACHED_NC = nc
    return _CACHED_NC


def host_route(coords_m, feats_m):
    """Pack mask coords into linear rows, drop not-found points, sort, and
    bucket per (core, partition).  Returns per-core input dicts (minus feats)."""
    c = coords_m.astype(np.int64)
    row = c[:, 1] * 16384 + c[:, 2] * 128 + c[:, 3]
    found = (
        (c[:, 0] == 0)
        & (c[:, 1] >= 0)
        & (c[:, 2] >= 0)
        & (c[:, 2] < 128)
        & (c[:, 3] >= 0)
        & (c[:, 3] < 128)
        & (row >= 0)
        & (row < NX)
    )
    rows = row[found]
    vals = feats_m[found, 0].astype(np.float32)
    order = np.argsort(rows, kind="stable")
    rows = rows[order]
    vals = vals[order]

    core_bounds = np.searchsorted(rows, np.arange(NCORES + 1) * RPC)
    per_core = []
    for cid in range(NCORES):
        r = rows[core_bounds[cid] : core_bounds[cid + 1]] - cid * RPC
        v = vals[core_bounds[cid] : core_bounds[cid + 1]]
        bnd = np.searchsorted(r, np.arange(129) * E)
        cnt = np.diff(bnd)
        if cnt.max() > C:
            # pathological duplicate pile-up: pre-merge duplicates on host
            ur, inv = np.unique(r, return_inverse=True)
            uv = np.zeros(len(ur), np.float32)
            np.add.at(uv, inv, v)
            r, v = ur, uv
            bnd = np.searchsorted(r, np.arange(129) * E)
            cnt = np.diff(bnd)
            assert cnt.max() <= C
        rbuf = np.full((128, C + 2), PAD, np.float32)
        rbuf[:, 0] = PRE
        rbuf[:, C + 1] = SUF
        vbuf = np.zeros((128, C), np.float32)
        if len(r):
            p_ids = np.repeat(np.arange(128), cnt)
            col = np.arange(len(r)) - bnd[:-1][p_ids]
            rbuf[p_ids, col + 1] = r.astype(np.float32)
            vbuf[p_ids, col] = v
        per_core.append({"rows": rbuf, "vals": vbuf})
    return per_core


def kernel(coords_x, feats_x, coords_m, feats_m):
    nc = _get_program()
    per_core = host_route(np.asarray(coords_m), np.asarray(feats_m))
    feats_x = np.ascontiguousarray(np.asarray(feats_x, dtype=np.float32))
    pbase = (np.arange(128, dtype=np.float32) * E).reshape(128, 1)

    in_maps = []
    for cid in range(NCORES):
        base = cid * RPC
        if base + PADR <= NX:
            fshard = feats_x[base : base + PADR]
        else:
            fshard = np.zeros((PADR, CF), np.float32)
            fshard[: NX - base] = feats_x[base:NX]
        in_maps.append(
            {
                "feats": fshard,
                "rows": per_core[cid]["rows"],
                "vals": per_core[cid]["vals"],
                "pbase": pbase,
            }
        )

    res = run_bass_kernel_spmd(nc, in_maps, core_ids=list(range(NCORES)))
    x_pruned = np.concatenate([res.results[c]["out"][:RPC] for c in range(NCORES)])
    target = np.concatenate([res.results[c]["targ"][:RPC] for c in range(NCORES)])
    return x_pruned, target.astype(bool)


if __name__ == "__main__":
    # quick self-exercise with random data
    rng = np.random.default_rng(0)
    i = np.arange(NX)
    coords_x = np.stack(
        [np.zeros_like(i), i // 16384, (i // 128) % 128, i % 128], axis=1
    ).astype(np.int32)
    feats_x = rng.standard_normal((NX, CF), dtype=np.float32)
    midx = rng.integers(0, NX, size=1_000_000)
    coords_m = coords_x[midx]
    feats_m = (rng.random((1_000_000, 1), dtype=np.float32) * 2).astype(np.float32)
    xp, tg = kernel(coords_x=coords_x, feats_x=feats_x, coords_m=coords_m, feats_m=feats_m)
    acc = np.zeros(NX, np.float32)
    np.add.at(acc, midx, feats_m[:, 0])
    ref_t = acc.astype(np.int32).astype(bool)
    ref_x = np.where(ref_t[:, None], feats_x, 0)
    print("target mismatches:", int((tg != ref_t).sum()))
    print("x rel err:", np.linalg.norm(xp - ref_x) / np.linalg.norm(ref_x))


# revision 11
# speedup vs baseline: 15.2781x; 15.2781x over previous
"""Trainium2 Bass kernel for nn_AttentionMask (scatter_memory).

Semantics (from the reference):
  - coords_x is the deterministic lexicographic grid (batch 0, z = i//16384,
    y = (i//128)%128, x = i%128) so the packed hash key of row i is strictly
    increasing in i: searchsorted(keys_x_sorted, key_m) degenerates to the
    linear index row = z*16384 + y*128 + x, and "found" is a simple range
    check on the coordinate fields.
  - mask scores are scatter-added onto their x rows; a row survives iff the
    accumulated score truncates (float->int) to nonzero, i.e. sum >= 1.0
    (scores are non-negative).
  - output = (feats_x zero-masked by survival, survival mask as bool).

Distribution: rows of x are sharded across 8 cores (250k rows each).  The
host packs+sorts the mask points by target row (as blessed by the problem's
sharding hint: "broadcast the sorted coordinate hash keys of the mask") and
routes each core/partition its slice.  Each core then, on device:
  1. merges duplicate rows with a segmented scan (DVE tensor_tensor_scan:
     state = same_as_prev*state + val, so the run-sum sits at each run tail),
  2. scatters the run-sums into a dense per-partition accumulator with the
     GPSIMD local_scatter vector-scatter op (f32 values travel as two uint16
     bit-planes since local_scatter moves 2-byte elements),
  3. thresholds (>= 1.0) to build the survival mask,
  4. streams feats_x through SBUF multiplying by the broadcast mask.
"""

import numpy as np

import concourse.bass as bass
import concourse.mybir as mybir
import concourse.tile as tile
from concourse import bacc
from concourse.bass_utils import run_bass_kernel_spmd

NX = 2_000_000
CF = 16
NCORES = 8
RPC = 250_000          # real rows per core
E = 1954               # accumulator elements per partition (128*E = 250112 >= RPC)
PADR = 128 * E         # padded rows per core
C = 1280               # point capacity per partition (covers uniform-ish inputs;
                       # kernel() falls back to a C=2048 build if data overflows)

PRE = 65000            # sentinel before column 0 (forces a run head)
PAD = 65001            # padding "offset" for unused point slots; also used as the
                       # suffix sentinel so trailing pads never produce a run tail

F32 = mybir.dt.float32
I16 = mybir.dt.int16
U16 = mybir.dt.uint16
U8 = mybir.dt.uint8


NCHUNKS = 8            # DMA chunks for the feats stream
FBUFS = 5              # feats tile pool buffers
SPLIT_RINGS = True     # inputs on SP HWDGE ring, outputs on ACT HWDGE ring


def _chunks():
    # split E row-groups into NCHUNKS DMA chunks of ~equal size
    base = E // NCHUNKS
    rem = E - base * NCHUNKS
    out = []
    g0 = 0
    for i in range(NCHUNKS):
        rg = base + (1 if i < rem else 0)
        out.append((g0, rg))
        g0 += rg
    return out


def build_device_program(nc, tc, ctx, C):
    feats = nc.dram_tensor("feats", [PADR, CF], F32, kind="ExternalInput")
    rows_p = nc.dram_tensor("rows", [128, C + 2], U16, kind="ExternalInput")
    vals_p = nc.dram_tensor("vals", [128, C], F32, kind="ExternalInput")
    outf = nc.dram_tensor("out", [PADR, CF], F32, kind="ExternalOutput")
    targ = nc.dram_tensor("targ", [PADR], U8, kind="ExternalOutput")

    pool = ctx.enter_context(tc.tile_pool(name="prep", bufs=1))
    fpool = ctx.enter_context(tc.tile_pool(name="feats", bufs=FBUFS))
    out_eng = nc.scalar if SPLIT_RINGS else nc.sync

    # "rows" holds per-partition row OFFSETS (0..E) as u16, with sentinel
    # columns at 0 and C+1; equal offset <=> equal global row since runs
    # never cross partitions.
    rows_t = pool.tile([128, C + 2], U16)
    vals_t = pool.tile([128, C], F32)
    nc.sync.dma_start(rows_t[:], rows_p[:])
    nc.sync.dma_start(vals_t[:], vals_p[:])

    r_cur = rows_t[:, 1 : C + 1]

    # same-as-previous flags and run-tail flags
    m_t = pool.tile([128, C], F32)
    nc.vector.tensor_tensor(m_t[:], r_cur, rows_t[:, 0:C], mybir.AluOpType.is_equal)

    # segmented run sums: state = m*state + val  (run total lands on the tail)
    s_t = pool.tile([128, C], F32)
    nc.vector.tensor_tensor_scan(
        s_t[:], m_t[:], vals_t[:], 0.0, mybir.AluOpType.mult, mybir.AluOpType.add
    )
    # survival flag per point position: run-sum >= 1.0 (only tails matter)
    flag_t = pool.tile([128, C], U16)
    nc.vector.tensor_scalar(flag_t[:], s_t[:], 1.0, None, mybir.AluOpType.is_ge)

    # scatter index: the offset at run tails, -1 elsewhere
    t_t = pool.tile([128, C], F32)
    nc.vector.tensor_tensor(
        t_t[:], r_cur, rows_t[:, 2 : C + 2], mybir.AluOpType.not_equal
    )
    idx_f = pool.tile([128, C], F32)
    nc.vector.tensor_scalar(idx_f[:], r_cur, 1.0, None, mybir.AluOpType.add)
    nc.vector.tensor_tensor(idx_f[:], idx_f[:], t_t[:], mybir.AluOpType.mult)
    nc.vector.tensor_scalar(
        idx_f[:], idx_f[:], -1.0, -1.0, mybir.AluOpType.add, mybir.AluOpType.max
    )
    idx_t = pool.tile([128, C], I16)
    nc.vector.tensor_copy(idx_t[:], idx_f[:])

    # dense per-partition scatter of the survival flags (zeroes the dest)
    dst_f = pool.tile([128, E], U16)
    nc.gpsimd.local_scatter(dst_f[:], flag_t[:], idx_t[:], 128, E, C)

    mask_t = pool.tile([128, E], F32)
    nc.vector.tensor_copy(mask_t[:], dst_f[:])
    targ_t = pool.tile([128, E], U8)
    nc.vector.tensor_copy(targ_t[:], dst_f[:])
    out_eng.dma_start(targ[:].rearrange("(p n) -> p n", p=128), targ_t[:])

    # main memory-bound loop: out = feats * mask (mask broadcast over CF)
    fview = feats[:].rearrange("(p n) f -> p n f", p=128)
    oview = outf[:].rearrange("(p n) f -> p n f", p=128)
    maxrg = max(rg for _, rg in _chunks())
    for g0, rg in _chunks():
        ft_full = fpool.tile([128, maxrg, CF], F32, tag="ft")
        ft = ft_full[:, :rg, :]
        nc.sync.dma_start(ft, fview[:, g0 : g0 + rg, :])
        mb = mask_t[:, g0 : g0 + rg].unsqueeze(2).to_broadcast([128, rg, CF])
        nc.vector.tensor_tensor(ft, ft, mb, mybir.AluOpType.mult)
        out_eng.dma_start(oview[:, g0 : g0 + rg, :], ft)


_CACHED = {}


def _get_program(cap=None):
    cap = C if cap is None else cap
    if cap not in _CACHED:
        from contextlib import ExitStack

        nc = bacc.Bacc(
            "TRN2", target_bir_lowering=False, debug=False, num_devices=NCORES
        )
        with tile.TileContext(nc) as tc:
            with ExitStack() as ctx:
                build_device_program(nc, tc, ctx, cap)
        nc.compile()
        _CACHED[cap] = nc
    return _CACHED[cap]


def _grid_is_canonical(coords_x):
    """Spot-check that coords_x is the deterministic lexicographic grid."""
    if coords_x is None or coords_x.shape != (NX, 4):
        return False
    idx = np.linspace(0, NX - 1, 1024).astype(np.int64)
    c = coords_x[idx].astype(np.int64)
    return bool(
        np.all(c[:, 0] == 0)
        and np.all(c[:, 1] == idx // 16384)
        and np.all(c[:, 2] == (idx // 128) % 128)
        and np.all(c[:, 3] == idx % 128)
    )


def _pack64(c):
    c = c.astype(np.int64)
    return (c[:, 0] << 48) | (c[:, 1] << 32) | (c[:, 2] << 16) | c[:, 3]


def host_route(coords_m, feats_m, coords_x=None):
    """Pack mask coords into linear rows, drop not-found points, sort, and
    bucket per (core, partition).  Returns per-core input dicts (minus feats)."""
    if coords_x is None or _grid_is_canonical(np.asarray(coords_x)):
        # canonical grid: hash-key order == row order, found == range check
        c = coords_m.astype(np.int64)
        row = c[:, 1] * 16384 + c[:, 2] * 128 + c[:, 3]
        found = (
            (c[:, 0] == 0)
            & (c[:, 1] >= 0)
            & (c[:, 2] >= 0)
            & (c[:, 2] < 128)
            & (c[:, 3] >= 0)
            & (c[:, 3] < 128)
            & (row >= 0)
            & (row < NX)
        )
    else:
        # general fallback: full key match per the reference semantics
        keys_x = _pack64(np.asarray(coords_x))
        keys_m = _pack64(coords_m)
        order = np.argsort(keys_x, kind="stable")
        sk = keys_x[order]
        pos = np.clip(np.searchsorted(sk, keys_m), 0, NX - 1)
        found = sk[pos] == keys_m
        row = order[pos]
    rows = row[found]
    vals = feats_m[found, 0].astype(np.float32)
    order = np.argsort(rows, kind="stable")
    rows = rows[order]
    vals = vals[order]

    core_bounds = np.searchsorted(rows, np.arange(NCORES + 1) * RPC)
    sliced = []
    need = 0
    for cid in range(NCORES):
        r = rows[core_bounds[cid] : core_bounds[cid + 1]] - cid * RPC
        v = vals[core_bounds[cid] : core_bounds[cid + 1]]
        bnd = np.searchsorted(r, np.arange(129) * E)
        cnt = np.diff(bnd)
        if cnt.max() > C:
            # pathological duplicate pile-up: pre-merge duplicates on host
            ur, inv = np.unique(r, return_inverse=True)
            uv = np.zeros(len(ur), np.float32)
            np.add.at(uv, inv, v.astype(np.float32))
            r, v = ur, uv
            bnd = np.searchsorted(r, np.arange(129) * E)
            cnt = np.diff(bnd)
        need = max(need, int(cnt.max()))
        sliced.append((r, v, bnd, cnt))

    cap = C if need <= C else 2048
    assert need <= cap, f"per-partition point count {need} exceeds capacity"
    per_core = []
    for r, v, bnd, cnt in sliced:
        rbuf = np.full((128, cap + 2), PAD, np.uint16)
        rbuf[:, 0] = PRE
        vbuf = np.zeros((128, cap), np.float32)
        if len(r):
            p_ids = np.repeat(np.arange(128), cnt)
            col = np.arange(len(r)) - bnd[:-1][p_ids]
            rbuf[p_ids, col + 1] = (r - p_ids * E).astype(np.uint16)
            vbuf[p_ids, col] = v
        per_core.append({"rows": rbuf, "vals": vbuf})
    return per_core, cap


def kernel(coords_x, feats_x, coords_m, feats_m):
    per_core, cap = host_route(np.asarray(coords_m), np.asarray(feats_m), coords_x)
    nc = _get_program(cap)
    feats_x = np.ascontiguousarray(np.asarray(feats_x, dtype=np.float32))

    in_maps = []
    for cid in range(NCORES):
        base = cid * RPC
        if base + PADR <= NX:
            fshard = feats_x[base : base + PADR]
        else:
            fshard = np.zeros((PADR, CF), np.float32)
            fshard[: NX - base] = feats_x[base:NX]
        in_maps.append(
            {
                "feats": fshard,
                "rows": per_core[cid]["rows"],
                "vals": per_core[cid]["vals"],
            }
        )

    res = run_bass_kernel_spmd(nc, in_maps, core_ids=list(range(NCORES)))
    x_pruned = np.concatenate([res.results[c]["out"][:RPC] for c in range(NCORES)])
    target = np.concatenate([res.results[c]["targ"][:RPC] for c in range(NCORES)])
    return x_pruned, target.astype(bool)


if __name__ == "__main__":
    # quick self-exercise with random data
    rng = np.random.default_rng(0)
    i = np.arange(NX)
    coords_x = np.stack(
        [np.zeros_like(i), i // 16384, (i // 128) % 128, i % 128], axis=1
    ).astype(np.int32)
    feats_x = rng.standard_normal((NX, CF), dtype=np.float32)
    midx = rng.integers(0, NX, size=1_000_000)
    coords_m = coords_x[midx]
    feats_m = (rng.random((1_000_000, 1), dtype=np.float32) * 2).astype(np.float32)
    xp, tg = kernel(coords_x=coords_x, feats_x=feats_x, coords_m=coords_m, feats_m=feats_m)
    acc = np.zeros(NX, np.float32)
    np.add.at(acc, midx, feats_m[:, 0])
    ref_t = acc.astype(np.int32).astype(bool)
    ref_x = np.where(ref_t[:, None], feats_x, 0)
    print("target mismatches:", int((tg != ref_t).sum()))
    print("x rel err:", np.linalg.norm(xp - ref_x) / np.linalg.norm(ref_x))
